# revision 38
# baseline (speedup 1.0000x reference)
"""Trainium2 Bass kernel for nn_Blur (upfirdn2d 4x4 blur, pad=(2,1)).

Formulation: out[i,j] = sum_{p,q} Kf[p,q] * x[i+p-2, j+q-2]   (Kf = flip(kernel2d))

For each W-tap q (4 taps), the H-convolution is a banded 64x64 matrix
Aq[i,h] = Kf[h-i+2, q].  Tolerance is 2e-2, so x streams as a single bf16
(the {1,3,9}/64 blur weights have <=4 mantissa bits: every bf16 product is
exact in fp32; end-to-end error ~5e-3) -- HALF the HBM traffic of an
fp32-faithful hi/lo split.

The K=128 contraction is filled by stacking TWO images per partition set:
lhsT_q = blockdiag(Aq^T, Aq^T) [128,128], rhs = [x_even; x_odd] [128, N],
so each matmul computes both images' H-conv at M=128 (full PE width, no
tile_position games).  The 4 taps accumulate into one PSUM bank with
variable-width windows: tap q=2 covers the full width first (start=True
initializes the per-element has_written state everywhere), the narrower
boundary taps then accumulate into column subsets.  This keeps the PE
~90% busy, which holds the clock-boost (HAM) state -- schemes with less
tensor work demote the PE clock to 1.2 GHz and end up slower.

The fp32 PSUM result is copied to SBUF as bf16 (alternating vector /
scalar engines), DMA'd back as [128,512] bf16 tiles, and cast to f32 on
the host.  HBM per core: 8.4 MB in + 8.4 MB out = the ~47us roofline.

Sharding: the 16*512 = 8192 independent (n,c) images are split into 8
contiguous slabs of 1024 images, one per NeuronCore (data-parallel).
"""

import ml_dtypes
import numpy as np

import concourse.bacc as bacc
import concourse.bass as bass
import concourse.mybir as mybir
import concourse.tile as tile
from concourse.bass_utils import run_bass_kernel_spmd

N_CORES = 8
IMG = 64                      # H = W
N_IMAGES = 16 * 512           # 8192
PER_CORE = N_IMAGES // N_CORES  # 1024
GROUP = 16                    # images per batch (8 pairs stacked in K)
N_BATCH = PER_CORE // GROUP   # 64
N_SUPER = N_BATCH // 4        # 16 DMA supertiles (4 batches each)
TILE_W = 8 * IMG              # 512 free cols per batch: 8 image pairs
CELL = IMG + 4                # 68-col input cells: 2 zero | 64 data | 2 zero
BTILE_W = 8 * CELL            # 544 input cols per batch
# per-tap W windows: tap q reads x cols [XLO[q], +LEN[q]) and writes out
# cols [JLO[q], +LEN[q]).  q=2 goes first: full width, start=True.
# Separable kernel: A1 == A2 == (3/8)Ah and A0 == A3 == (1/8)Ah, so taps
# are ordered (2,1,0,3) and adjacent taps share the SAME stationary
# weights AP -- 2 weight loads per batch instead of 4.
TAP_ORDER = (2, 1, 0, 3)
WBLK = (1, 0, 0, 1)           # weight block per tap: 0 = 3/8 Ah, 1 = 1/8 Ah
XLO = (0, 0, 0, 1)
JLO = (2, 1, 0, 0)
LEN = (62, 63, 64, 63)
DT = mybir.dt.float32
IN_DT = mybir.dt.bfloat16
NP_IN = ml_dtypes.bfloat16

LAST_RESULTS = None  # BassKernelResults of the most recent run (for test.py)


def _build_weights(kernel2d: np.ndarray) -> np.ndarray:
    """[128, 256] bf16: block m holds blockdiag(Aq^T, Aq^T) where block 0
    serves taps 1,2 (== 3/8 Ah) and block 1 serves taps 0,3 (== 1/8 Ah)."""
    kf = np.flip(np.asarray(kernel2d, dtype=np.float64), (0, 1))
    wts = np.zeros((128, 256), dtype=NP_IN)
    for q, m in ((2, 0), (0, 1)):  # tap q is representative of block m
        aq = np.zeros((IMG, IMG), dtype=np.float64)
        for i in range(IMG):
            for p in range(4):
                h = i + p - 2
                if 0 <= h < IMG:
                    aq[i, h] = kf[p, q]
        aqt = aq.T.astype(NP_IN)
        wts[:IMG, 128 * m : 128 * m + IMG] = aqt
        wts[IMG:, 128 * m + IMG : 128 * m + 128] = aqt
    return wts


def _bass_module() -> bass.Bass:
    nc = bacc.Bacc(
        "TRN2",
        target_bir_lowering=False,
        debug=False,
        num_devices=N_CORES,
    )
    x_d = nc.dram_tensor(
        "x", [N_SUPER, 128, 4 * BTILE_W], IN_DT, kind="ExternalInput"
    )
    w_d = nc.dram_tensor("wts", [128, 256], IN_DT, kind="ExternalInput")
    o_d = nc.dram_tensor(
        "out", [N_SUPER, 128, 4 * 512], IN_DT, kind="ExternalOutput"
    )

    with tile.TileContext(nc) as tc:
        with (
            tc.tile_pool(name="const", bufs=1) as cpool,
            tc.tile_pool(name="inp", bufs=5) as ipool,
            tc.tile_pool(name="outp", bufs=3) as opool,
            tc.tile_pool(name="uv", bufs=6) as uvpool,
            tc.tile_pool(name="psum", bufs=8, space="PSUM") as ppool,
        ):
            w_tile = cpool.tile([128, 256], IN_DT)
            # first weight block lands first (32KB) so warmup can start
            # ~2us earlier; the rest follows while warmup runs.
            nc.sync.dma_start(w_tile[:, 0:128], w_d[:, 0:128])
            nc.sync.dma_start(w_tile[:, 128:256], w_d[:, 128:256])

            # HAM warmup: the PE clock-gate holds 1.2 GHz until ~3.4us of
            # sustained matmul activity.  Burn that window on dummy matmuls
            # reading the (just-DMA'd) first weight block as both operands.
            warm_ps = ppool.tile([128, 512], DT, tag="ps")
            for _ in range(16):
                nc.tensor.matmul(
                    warm_ps[:, 0:256],
                    w_tile[:, 0:128],
                    w_tile[:],
                    start=True,
                    stop=True,
                )

            # DMA granularity: every dma_start costs ~600ns of descriptor
            # generation on its issuing sequencer (software DGE), so DMAs
            # move 4 batches at a time (supertiles); compute, PSUM and
            # copies stay at single-batch granularity.
            LOOKAHEAD = 4
            in_tiles = {}

            def issue_in(s):
                t = ipool.tile([128, 4 * BTILE_W], IN_DT)
                nc.sync.dma_start(t[:], x_d[s])
                in_tiles[s] = t

            for s in range(LOOKAHEAD):
                issue_in(s)

            out_super = None
            for b in range(N_BATCH):
                s, k = divmod(b, 4)
                if k == 0:
                    if s + LOOKAHEAD < N_SUPER:
                        issue_in(s + LOOKAHEAD)
                    in_super = in_tiles.pop(s)
                    out_super = opool.tile([128, 4 * 512], IN_DT)
                x3 = in_super[:, k * BTILE_W : (k + 1) * BTILE_W].rearrange(
                    "p (g w) -> p g w", w=CELL
                )
                # W-conv as two pairwise sums on the DVE-class engines:
                #   v[j] = x[j-1]+x[j]    (weights 3/8 Ah)
                #   u[j] = x[j-2]+x[j+1]  (weights 1/8 Ah)
                # The 2+2 zero pad cols in each 68-col cell make every
                # out-of-range tap read an exact zero -- no windows, no
                # boundary fixups, and the tensor engine streams only TWO
                # full-width matmuls per batch (1024 cols vs 2016).
                v_t = uvpool.tile([128, 512], IN_DT)
                u_t = uvpool.tile([128, 512], IN_DT)
                v3 = v_t[:].rearrange("p (g w) -> p g w", w=IMG)
                u3 = u_t[:].rearrange("p (g w) -> p g w", w=IMG)
                nc.vector.tensor_add(v3, x3[:, :, 1:65], x3[:, :, 2:66])
                nc.gpsimd.tensor_add(u3, x3[:, :, 0:64], x3[:, :, 3:67])

                ps = ppool.tile([128, 512], DT)
                nc.tensor.matmul(
                    ps[:], w_tile[:, 0:128], v_t[:], start=True, stop=False
                )
                nc.tensor.matmul(
                    ps[:], w_tile[:, 128:256], u_t[:], start=False, stop=True
                )
                # keep-alive dummy: holds the HAM duty fraction above the
                # demote threshold now that real tensor work is halved
                nc.tensor.matmul(
                    warm_ps[:, 0:256],
                    w_tile[:, 0:128],
                    w_tile[:],
                    start=True,
                    stop=True,
                )

                dst = out_super[:, k * 512 : (k + 1) * 512]
                if b % 2 == 0:
                    nc.vector.tensor_copy(dst, ps[:])
                else:
                    nc.scalar.copy(dst, ps[:])
                if s == N_SUPER - 1:
                    # final supertile: per-batch out-DMAs overlap the last
                    # copies instead of waiting for all four
                    eng = nc.sync if k % 2 == 0 else nc.scalar
                    eng.dma_start(
                        o_d[s][:, k * 512 : (k + 1) * 512], dst
                    )
                elif k == 3:
                    # alternate the issuing sequencer per supertile
                    eng = nc.sync if s % 2 == 0 else nc.scalar
                    eng.dma_start(o_d[s], out_super[:])
    nc.compile()
    return nc


def _host_pack(x: np.ndarray) -> np.ndarray:
    """FULL x (8192,64,64) f32 -> [N_CORES, N_BATCH, 128, 512] bf16.

    Partition dim = (a, h), a = image parity in pair; free dim = (g: 8
    pairs, w).  img = core*1024 + b*16 + g*2 + a."""
    v = x.reshape(N_CORES, N_BATCH, 8, 2, IMG, IMG).transpose(0, 1, 3, 4, 2, 5)
    arr = np.zeros((N_CORES, N_BATCH, 2, IMG, 8, CELL), dtype=NP_IN)
    arr[..., 2 : 2 + IMG] = v.astype(NP_IN)
    flat = arr.reshape(N_CORES, N_SUPER, 4, 128, BTILE_W)
    return np.ascontiguousarray(flat.transpose(0, 1, 3, 2, 4)).reshape(
        N_CORES, N_SUPER, 128, 4 * BTILE_W
    )


def _host_unpack(tiles: np.ndarray) -> np.ndarray:
    """[N_CORES, N_BATCH, 128, 512] bf16 -> (8192, 64, 64) f32."""
    v = tiles.reshape(N_CORES, N_SUPER, 128, 4, 512)
    v = v.transpose(0, 1, 3, 2, 4).reshape(N_CORES, N_BATCH, 2, IMG, 8, IMG)
    v = v.transpose(0, 1, 4, 2, 3, 5)  # [core, b, g, a, i, j]
    return v.reshape(N_IMAGES, IMG, IMG).astype(np.float32)


def kernel(x: np.ndarray, kernel: np.ndarray, _trace: bool = False) -> np.ndarray:
    global LAST_RESULTS
    x = np.ascontiguousarray(np.asarray(x, dtype=np.float32))
    n, c, h, w = x.shape
    assert (n, c, h, w) == (16, 512, 64, 64), x.shape

    shards = _host_pack(x.reshape(N_IMAGES, IMG, IMG))
    wts = _build_weights(kernel)
    in_maps = [{"x": shards[i], "wts": wts} for i in range(N_CORES)]

    nc = _bass_module()
    results = run_bass_kernel_spmd(
        nc, in_maps, core_ids=list(range(N_CORES)), trace=_trace
    )
    LAST_RESULTS = results

    tiles = np.stack([r["out"] for r in results.results])
    out = _host_unpack(tiles)
    return np.ascontiguousarray(out.reshape(n, c, h, w)).astype(np.float32)


# revision 39
# speedup vs baseline: 1.3116x; 1.3116x over previous
"""Trainium2 Bass kernel for nn_Blur (upfirdn2d 4x4 blur, pad=(2,1)).

Formulation: out[i,j] = sum_{p,q} Kf[p,q] * x[i+p-2, j+q-2]   (Kf = flip(kernel2d))

For each W-tap q (4 taps), the H-convolution is a banded 64x64 matrix
Aq[i,h] = Kf[h-i+2, q].  Tolerance is 2e-2, so x streams as a single bf16
(the {1,3,9}/64 blur weights have <=4 mantissa bits: every bf16 product is
exact in fp32; end-to-end error ~5e-3) -- HALF the HBM traffic of an
fp32-faithful hi/lo split.

The K=128 contraction is filled by stacking TWO images per partition set:
lhsT_q = blockdiag(Aq^T, Aq^T) [128,128], rhs = [x_even; x_odd] [128, N],
so each matmul computes both images' H-conv at M=128 (full PE width, no
tile_position games).  The 4 taps accumulate into one PSUM bank with
variable-width windows: tap q=2 covers the full width first (start=True
initializes the per-element has_written state everywhere), the narrower
boundary taps then accumulate into column subsets.  This keeps the PE
~90% busy, which holds the clock-boost (HAM) state -- schemes with less
tensor work demote the PE clock to 1.2 GHz and end up slower.

The fp32 PSUM result is copied to SBUF as bf16 (alternating vector /
scalar engines), DMA'd back as [128,512] bf16 tiles, and cast to f32 on
the host.  HBM per core: 8.4 MB in + 8.4 MB out = the ~47us roofline.

Sharding: the 16*512 = 8192 independent (n,c) images are split into 8
contiguous slabs of 1024 images, one per NeuronCore (data-parallel).
"""

import ml_dtypes
import numpy as np

import concourse.bacc as bacc
import concourse.bass as bass
import concourse.mybir as mybir
import concourse.tile as tile
from concourse.bass_utils import run_bass_kernel_spmd

N_CORES = 8
IMG = 64                      # H = W
N_IMAGES = 16 * 512           # 8192
PER_CORE = N_IMAGES // N_CORES  # 1024
GROUP = 16                    # images per batch (8 pairs stacked in K)
N_BATCH = PER_CORE // GROUP   # 64
N_SUPER = N_BATCH // 4        # 16 DMA supertiles (4 batches each)
TILE_W = 8 * IMG              # 512 free cols per batch: 8 image pairs
# per-tap W windows: tap q reads x cols [XLO[q], +LEN[q]) and writes out
# cols [JLO[q], +LEN[q]).  q=2 goes first: full width, start=True.
# Separable kernel: A1 == A2 == (3/8)Ah and A0 == A3 == (1/8)Ah, so taps
# are ordered (2,1,0,3) and adjacent taps share the SAME stationary
# weights AP -- 2 weight loads per batch instead of 4.
TAP_ORDER = (2, 1, 0, 3)
WBLK = (1, 0, 0, 1)           # weight block per tap: 0 = 3/8 Ah, 1 = 1/8 Ah
XLO = (0, 0, 0, 1)
JLO = (2, 1, 0, 0)
LEN = (62, 63, 64, 63)
DT = mybir.dt.float32
IN_DT = mybir.dt.bfloat16
NP_IN = ml_dtypes.bfloat16

LAST_RESULTS = None  # BassKernelResults of the most recent run (for test.py)


def _build_weights(kernel2d: np.ndarray) -> np.ndarray:
    """[128, 256] bf16: block m holds blockdiag(Aq^T, Aq^T) where block 0
    serves taps 1,2 (== 3/8 Ah) and block 1 serves taps 0,3 (== 1/8 Ah)."""
    kf = np.flip(np.asarray(kernel2d, dtype=np.float64), (0, 1))
    wts = np.zeros((128, 256), dtype=NP_IN)
    for q, m in ((2, 0), (0, 1)):  # tap q is representative of block m
        aq = np.zeros((IMG, IMG), dtype=np.float64)
        for i in range(IMG):
            for p in range(4):
                h = i + p - 2
                if 0 <= h < IMG:
                    aq[i, h] = kf[p, q]
        aqt = aq.T.astype(NP_IN)
        wts[:IMG, 128 * m : 128 * m + IMG] = aqt
        wts[IMG:, 128 * m + IMG : 128 * m + 128] = aqt
    return wts


def _bass_module() -> bass.Bass:
    nc = bacc.Bacc(
        "TRN2",
        target_bir_lowering=False,
        debug=False,
        num_devices=N_CORES,
    )
    x_d = nc.dram_tensor(
        "x", [N_SUPER, 128, 4 * TILE_W], IN_DT, kind="ExternalInput"
    )
    w_d = nc.dram_tensor("wts", [128, 256], IN_DT, kind="ExternalInput")
    o_d = nc.dram_tensor(
        "out", [N_SUPER, 128, 4 * 512], IN_DT, kind="ExternalOutput"
    )

    with tile.TileContext(nc) as tc:
        with (
            tc.tile_pool(name="const", bufs=1) as cpool,
            tc.tile_pool(name="inp", bufs=5) as ipool,
            tc.tile_pool(name="outp", bufs=3) as opool,
            tc.tile_pool(name="psum", bufs=8, space="PSUM") as ppool,
        ):
            w_tile = cpool.tile([128, 256], IN_DT)
            # first weight block lands first (32KB) so warmup can start
            # ~2us earlier; the rest follows while warmup runs.
            nc.sync.dma_start(w_tile[:, 0:128], w_d[:, 0:128])
            nc.sync.dma_start(w_tile[:, 128:256], w_d[:, 128:256])

            # HAM warmup: the PE clock-gate holds 1.2 GHz until ~3.4us of
            # sustained matmul activity.  Burn that window on dummy matmuls
            # reading the (just-DMA'd) first weight block as both operands.
            warm_ps = ppool.tile([128, 512], DT, tag="ps")
            for _ in range(16):
                nc.tensor.matmul(
                    warm_ps[:, 0:256],
                    w_tile[:, 0:128],
                    w_tile[:],
                    start=True,
                    stop=True,
                )

            # DMA granularity: every dma_start costs ~600ns of descriptor
            # generation on its issuing sequencer (software DGE), so DMAs
            # move 4 batches at a time (supertiles); compute, PSUM and
            # copies stay at single-batch granularity.
            LOOKAHEAD = 4
            in_tiles = {}

            def issue_in(s):
                t = ipool.tile([128, 4 * TILE_W], IN_DT)
                nc.sync.dma_start(t[:], x_d[s])
                in_tiles[s] = t

            for s in range(LOOKAHEAD):
                issue_in(s)

            out_super = None
            for b in range(N_BATCH):
                s, k = divmod(b, 4)
                if k == 0:
                    if s + LOOKAHEAD < N_SUPER:
                        issue_in(s + LOOKAHEAD)
                    in_super = in_tiles.pop(s)
                    out_super = opool.tile([128, 4 * 512], IN_DT)
                rhs3 = in_super[:, k * TILE_W : (k + 1) * TILE_W].rearrange(
                    "p (g w) -> p g w", w=IMG
                )

                ps = ppool.tile([128, 512], DT)
                out3 = ps[:].rearrange("p (g w) -> p g w", w=IMG)
                for qi, q in enumerate(TAP_ORDER):
                    nc.tensor.matmul(
                        out3[:, :, JLO[q] : JLO[q] + LEN[q]],
                        w_tile[:, 128 * WBLK[q] : 128 * WBLK[q] + 128],
                        rhs3[:, :, XLO[q] : XLO[q] + LEN[q]],
                        start=(qi == 0),
                        stop=(qi == 3),
                    )

                dst = out_super[:, k * 512 : (k + 1) * 512]
                if b % 2 == 0:
                    nc.vector.tensor_copy(dst, ps[:])
                else:
                    nc.scalar.copy(dst, ps[:])
                if s == N_SUPER - 1:
                    # final supertile: per-batch out-DMAs overlap the last
                    # copies instead of waiting for all four
                    eng = nc.sync if k % 2 == 0 else nc.scalar
                    eng.dma_start(
                        o_d[s][:, k * 512 : (k + 1) * 512], dst
                    )
                elif k == 3:
                    # alternate the issuing sequencer per supertile
                    eng = nc.sync if s % 2 == 0 else nc.scalar
                    eng.dma_start(o_d[s], out_super[:])
    nc.compile()
    return nc


def _host_pack(x: np.ndarray) -> np.ndarray:
    """FULL x (8192,64,64) f32 -> [N_CORES, N_BATCH, 128, 512] bf16.

    Partition dim = (a, h), a = image parity in pair; free dim = (g: 8
    pairs, w).  img = core*1024 + b*16 + g*2 + a."""
    v = x.reshape(N_CORES, N_BATCH, 8, 2, IMG, IMG).transpose(0, 1, 3, 4, 2, 5)
    flat = np.ascontiguousarray(v.astype(NP_IN)).reshape(
        N_CORES, N_SUPER, 4, 128, TILE_W
    )
    return np.ascontiguousarray(flat.transpose(0, 1, 3, 2, 4)).reshape(
        N_CORES, N_SUPER, 128, 4 * TILE_W
    )


def _host_unpack(tiles: np.ndarray) -> np.ndarray:
    """[N_CORES, N_BATCH, 128, 512] bf16 -> (8192, 64, 64) f32."""
    v = tiles.reshape(N_CORES, N_SUPER, 128, 4, 512)
    v = v.transpose(0, 1, 3, 2, 4).reshape(N_CORES, N_BATCH, 2, IMG, 8, IMG)
    v = v.transpose(0, 1, 4, 2, 3, 5)  # [core, b, g, a, i, j]
    return v.reshape(N_IMAGES, IMG, IMG).astype(np.float32)


def kernel(x: np.ndarray, kernel: np.ndarray, _trace: bool = False) -> np.ndarray:
    global LAST_RESULTS
    x = np.ascontiguousarray(np.asarray(x, dtype=np.float32))
    n, c, h, w = x.shape
    assert (n, c, h, w) == (16, 512, 64, 64), x.shape

    shards = _host_pack(x.reshape(N_IMAGES, IMG, IMG))
    wts = _build_weights(kernel)
    in_maps = [{"x": shards[i], "wts": wts} for i in range(N_CORES)]

    nc = _bass_module()
    results = run_bass_kernel_spmd(
        nc, in_maps, core_ids=list(range(N_CORES)), trace=_trace
    )
    LAST_RESULTS = results

    tiles = np.stack([r["out"] for r in results.results])
    out = _host_unpack(tiles)
    return np.ascontiguousarray(out.reshape(n, c, h, w)).astype(np.float32)
